# revision 4
# baseline (speedup 1.0000x reference)
"""Distributed 3-layer GraphConv GNN on 8 Trainium2 NeuronCores — v2.

Sharding: nodes (and incoming edges) partitioned contiguously across 8 cores
(2560 nodes / 20 blocks of 128 dst nodes per core). All feature math in bf16
(f32 PSUM accumulation), which is well within the 2e-2 tolerance (measured
1.5e-3 end-to-end).

Per layer, each core:
  - dma_gathers source-node rows (bf16, 256B/edge) for its edges from the
    node-major DRAM copy of the previous layer's features, spreading the
    1024-index gather calls round-robin over 4 SWDGE queues so their
    ~2us completion latencies overlap instead of serializing,
  - segment-sums each 128-edge chunk into its dst block on the TensorEngine
    via host-precomputed one-hot matrices (edges are grouped by 32-wide dst
    ranges, so each chunk's one-hot is only [128, 32]; all one-hots live in
    SBUF for the whole kernel and are shared by the 3 layers),
  - applies W_rel/W_root + bias + ReLU (feature-major, bf16 out),
  - transposes to node-major and AllGathers (bf16, Shared output) so the
    next layer can gather from the full feature matrix.
Graph pooling uses a host-precomputed batch one-hot, is AllReduced (f32),
and every core redundantly computes logits + log_softmax.
"""
import sys

sys.path.insert(0, "/opt/trn_rl_repo")

from contextlib import ExitStack

import ml_dtypes
import numpy as np

import concourse.bass as bass
import concourse.tile as tile
from concourse import bacc, mybir
from concourse.bass_utils import run_bass_kernel_spmd
from concourse.library_config import mlp as mlp_lib

N, E, F_IN, H, C_OUT, G = 20000, 640000, 64, 128, 10, 128
NCORES = 8
NPC = 2560            # nodes per core
NBLK = NPC // 128     # dst blocks per core (20)
NPAD = NCORES * NPC   # 20480
W = 32                # dst-group width for one-hot segment sum
NG = 128 // W         # groups per block
GCH = 8               # chunks (of 128 edges) per dma_gather call (1024 idx max)
F32 = mybir.dt.float32
BF16 = mybir.dt.bfloat16
AF = mybir.ActivationFunctionType
ALU = mybir.AluOpType
BF = ml_dtypes.bfloat16


def _prep_inputs(x, edge_index, batch):
    """Host-side edge partitioning/padding + one-hot precompute.

    Returns (in_maps, struct) where struct = (cmax rows tuple) describing the
    uniform per-(block, group) chunk counts the SPMD program is built for.
    """
    src = np.asarray(edge_index[0], dtype=np.int64)
    dst = np.asarray(edge_index[1], dtype=np.int64)
    batch = np.asarray(batch, dtype=np.int64)
    x = np.ascontiguousarray(np.asarray(x, dtype=np.float32))

    order = np.argsort(dst, kind="stable")
    dst_s = dst[order]
    src_s = src[order]
    NGB = NCORES * NBLK
    bounds = np.searchsorted(dst_s, np.arange(NGB * NG + 1) * W)
    cnts = np.diff(bounds).reshape(NCORES, NBLK, NG)
    cmax = np.maximum(1, -(-cnts.max(axis=0) // 128))  # [NBLK, NG] chunks
    CB = cmax.sum(axis=1)                              # chunks per block
    TOTC = int(CB.sum())
    # global chunk offset of (b, j)
    gck = np.zeros((NBLK, NG), np.int64)
    flat = np.concatenate([[0], np.cumsum(cmax.reshape(-1))])[:-1]
    gck[:, :] = flat.reshape(NBLK, NG)
    cstart = np.concatenate([[0], np.cumsum(CB)])  # chunk start per block

    # one-hot m: same [128, TOTC*W] layout per core (content differs)
    x_pad = np.zeros((N, 128), np.float32)
    x_pad[:, :F_IN] = x

    in_maps = []
    for k in range(NCORES):
        idx_flat = np.zeros(TOTC * 128, np.int64)
        m = np.zeros((128, TOTC * W), np.float32)
        for b in range(NBLK):
            for j in range(NG):
                gi = (k * NBLK + b) * NG + j
                s, e = bounds[gi], bounds[gi + 1]
                n = e - s
                if n == 0:
                    continue
                pos = np.arange(n)
                idx_flat[gck[b, j] * 128 + pos] = src_s[s:e]
                drel = dst_s[s:e] - ((k * NBLK + b) * 128 + j * W)
                col = (gck[b, j] + pos // 128) * W + drel
                m[pos % 128, col] = 1.0
        idx16 = idx_flat.reshape(TOTC * 8, 16).T.astype(np.int16)
        idx_t = np.ascontiguousarray(np.tile(idx16, (8, 1)))

        base = k * NPC
        valid = max(0, min(NPC, N - base))
        pm = np.zeros((128, NBLK * 128), np.float32)
        gid = np.full(NPC, -1, np.int64)
        if valid:
            gid[:valid] = batch[base:base + valid]
        for b in range(NBLK):
            gb = gid[b * 128:(b + 1) * 128]
            ok = gb >= 0
            pm[np.arange(128)[ok], b * 128 + gb[ok]] = 1.0

        xT = np.zeros((128, NPC), np.float32)
        if valid:
            xT[:F_IN, :valid] = x[base:base + valid].T

        in_maps.append({
            "x_pad": x_pad.astype(BF),
            "idx_t": idx_t,
            "m_t": np.ascontiguousarray(m.astype(BF)),
            "pm_t": np.ascontiguousarray(pm.astype(BF)),
            "xT_t": np.ascontiguousarray(xT.astype(BF)),
            "ident_t": np.eye(128, dtype=BF),
            "ones_t": np.ones((1, 128), np.float32),
        })
    struct = tuple(map(tuple, cmax.tolist()))
    return in_maps, struct


def _build_program(struct, dbg=None, prep_mode=False, nq=4):
    cmax = np.asarray(struct, np.int64)          # [NBLK, NG]
    CB = cmax.sum(axis=1)
    TOTC = int(CB.sum())
    CBMAX = int(CB.max())
    cstart = np.concatenate([[0], np.cumsum(CB)])

    nc = bacc.Bacc("TRN2", target_bir_lowering=False, debug=False,
                   num_devices=NCORES, dynamic_dma_scratch_size=1 << 15,
                   num_swdge_queues=nq)

    x_pad = nc.dram_tensor("x_pad", [N, 128], BF16, kind="ExternalInput")
    idx_t = nc.dram_tensor("idx_t", [128, TOTC * 8], mybir.dt.int16,
                           kind="ExternalInput")
    m_t = nc.dram_tensor("m_t", [128, TOTC * W], BF16, kind="ExternalInput")
    pm_t = nc.dram_tensor("pm_t", [128, NBLK * 128], BF16,
                          kind="ExternalInput")
    xT_t = nc.dram_tensor("xT_t", [128, NPC], BF16, kind="ExternalInput")
    ident_t = nc.dram_tensor("ident_t", [128, 128], BF16,
                             kind="ExternalInput")
    ones_t = nc.dram_tensor("ones_t", [1, 128], F32, kind="ExternalInput")
    w_rel_in = [nc.dram_tensor(f"w{i}_rel", [128, H], BF16,
                               kind="ExternalInput") for i in (1, 2, 3)]
    w_root_in = [nc.dram_tensor(f"w{i}_root", [128, H], BF16,
                                kind="ExternalInput") for i in (1, 2, 3)]
    b_in = [nc.dram_tensor(f"b{i}", [H, 1], F32, kind="ExternalInput")
            for i in (1, 2, 3)]
    w_out_in = nc.dram_tensor("w_out", [H, C_OUT], F32, kind="ExternalInput")
    b_out_in = nc.dram_tensor("b_out", [1, C_OUT], F32, kind="ExternalInput")
    out_t = nc.dram_tensor("out", [G, C_OUT], F32, kind="ExternalOutput")
    dbg_t = (nc.dram_tensor("dbg", [128, NPC], F32, kind="ExternalOutput")
             if dbg else None)

    with tile.TileContext(nc) as tc, ExitStack() as ctx:
        const = ctx.enter_context(tc.tile_pool(name="const", bufs=1))
        feat = ctx.enter_context(tc.tile_pool(name="feat", bufs=1))
        xe_pool = ctx.enter_context(tc.tile_pool(name="xe", bufs=6))
        nm_pool = ctx.enter_context(tc.tile_pool(name="nm", bufs=3))
        sm_pool = ctx.enter_context(tc.tile_pool(name="sm", bufs=1))
        psA = ctx.enter_context(tc.tile_pool(name="psA", bufs=2, space="PSUM"))
        psB = ctx.enter_context(tc.tile_pool(name="psB", bufs=2, space="PSUM"))
        psT = ctx.enter_context(tc.tile_pool(name="psT", bufs=2, space="PSUM"))
        psP = ctx.enter_context(tc.tile_pool(name="psP", bufs=1, space="PSUM"))
        dram = ctx.enter_context(tc.tile_pool(name="dram", bufs=1,
                                              space="DRAM"))

        nc.gpsimd.load_library(mlp_lib)
        dma_sem = nc.alloc_semaphore("gsem")

        idx_sb = const.tile([128, TOTC * 8], mybir.dt.int16)
        nc.sync.dma_start(idx_sb[:], idx_t[:])
        m_sb = const.tile([128, TOTC * W], BF16)
        nc.sync.dma_start(m_sb[:], m_t[:])
        pm_sb = const.tile([128, NBLK * 128], BF16)
        nc.sync.dma_start(pm_sb[:], pm_t[:])
        ident_sb = const.tile([128, 128], BF16)
        nc.sync.dma_start(ident_sb[:], ident_t[:])
        ones_sb = const.tile([1, 128], F32)
        nc.sync.dma_start(ones_sb[:], ones_t[:])
        w_rel_sb, w_root_sb, b_sb = [], [], []
        for i in range(3):
            wr = const.tile([128, H], BF16, name=f"wrel{i}")
            nc.sync.dma_start(wr[:], w_rel_in[i][:])
            w_rel_sb.append(wr)
            wo = const.tile([128, H], BF16, name=f"wroot{i}")
            nc.sync.dma_start(wo[:], w_root_in[i][:])
            w_root_sb.append(wo)
            bb = const.tile([H, 1], F32, name=f"b{i}")
            nc.sync.dma_start(bb[:], b_in[i][:])
            b_sb.append(bb)
        wout_sb = const.tile([H, C_OUT], F32)
        nc.sync.dma_start(wout_sb[:], w_out_in[:])
        bout_sb = const.tile([1, C_OUT], F32)
        nc.sync.dma_start(bout_sb[:], b_out_in[:])

        xT_sb = feat.tile([128, NPC], BF16)
        nc.sync.dma_start(xT_sb[:], xT_t[:])
        h1T_sb = feat.tile([128, NPC], BF16)
        h2T_sb = feat.tile([128, NPC], BF16)
        h3T_sb = feat.tile([128, NPC], BF16)
        aggT_sb = feat.tile([128, NPC], BF16)
        nmall_sb = feat.tile([128, NPC], BF16)

        h1_loc = dram.tile([NPC, H], BF16)
        h2_loc = dram.tile([NPC, H], BF16)
        h1_full = dram.tile([NPAD, H], BF16, addr_space="Shared")
        h2_full = dram.tile([NPAD, H], BF16, addr_space="Shared")
        pool_in = dram.tile([H, G], F32)
        pool_out = dram.tile([H, G], F32, addr_space="Shared")

        def gcn_layer(li, gather_src, inT_sb, outT_sb, h_loc, h_full):
            wrel, wroot, bb = w_rel_sb[li], w_root_sb[li], b_sb[li]
            for b in range(NBLK):
                c0, c1 = int(cstart[b]), int(cstart[b + 1])
                nch = c1 - c0
                xe = xe_pool.tile([128, CBMAX * 128], BF16, tag="xe",
                                  name=f"xe{li}_{b}")
                for g0 in range(0, nch, GCH):
                    g1 = min(g0 + GCH, nch)
                    nsub = (g1 - g0) * 128
                    xe3 = xe[:, g0 * 128:g1 * 128].rearrange(
                        "p (c f) -> p c f", f=128)
                    if prep_mode:
                        nc.gpsimd.dma_gather(
                            xe3, gather_src[:],
                            idx_sb[:, (c0 + g0) * 8:(c0 + g0) * 8 + nsub // 16],
                            nsub, nsub, 128, prepare_only=True, sem=dma_sem)
                    else:
                        nc.gpsimd.dma_gather(
                            xe3, gather_src[:],
                            idx_sb[:, (c0 + g0) * 8:(c0 + g0) * 8 + nsub // 16],
                            nsub, nsub, 128,
                            queue_num=(b * 8 + g0 // GCH) % nq)
                if prep_mode:
                    nc.gpsimd.trigger_dma(count=None)
                agg_ps = psA.tile([128, 128], F32, tag="agg",
                                  name=f"agg{li}_{b}")
                ck = 0
                for j in range(NG):
                    nj = int(cmax[b, j])
                    for c in range(nj):
                        gc = c0 + ck
                        nc.tensor.matmul(
                            agg_ps[:, j * W:(j + 1) * W],
                            xe[:, ck * 128:(ck + 1) * 128],
                            m_sb[:, gc * W:(gc + 1) * W],
                            start=(c == 0), stop=(c == nj - 1))
                        ck += 1
                nc.vector.tensor_copy(
                    aggT_sb[:, b * 128:(b + 1) * 128], agg_ps[:])
            for g in range(NPC // 512):
                hp = psB.tile([H, 512], F32, tag="hp", name=f"hp{li}_{g}")
                nc.tensor.matmul(hp[:], wrel[:],
                                 aggT_sb[:, g * 512:(g + 1) * 512],
                                 start=True, stop=False)
                nc.tensor.matmul(hp[:], wroot[:],
                                 inT_sb[:, g * 512:(g + 1) * 512],
                                 start=False, stop=True)
                nc.scalar.activation(outT_sb[:, g * 512:(g + 1) * 512],
                                     hp[:], AF.Relu, bias=bb[:])
            if h_loc is not None:
                for b in range(NBLK):
                    tp = psT.tile([128, 128], BF16, tag="tp",
                                  name=f"tp{li}_{b}")
                    nc.tensor.transpose(
                        tp[:], outT_sb[:, b * 128:(b + 1) * 128], ident_sb[:])
                    nc.scalar.copy(nmall_sb[:, b * 128:(b + 1) * 128], tp[:])
                nc.sync.dma_start(
                    h_loc[:].rearrange("(b p) f -> p b f", p=128),
                    nmall_sb[:])
                nc.gpsimd.collective_compute(
                    "AllGather", ALU.bypass,
                    replica_groups=[list(range(NCORES))],
                    ins=[h_loc.opt()], outs=[h_full.opt()])

        gcn_layer(0, x_pad, xT_sb, h1T_sb, h1_loc, h1_full)
        gcn_layer(1, h1_full, h1T_sb, h2T_sb, h2_loc, h2_full)
        gcn_layer(2, h2_full, h2T_sb, h3T_sb, None, None)

        if dbg in ("agg1", "h1T", "nmall", "h2T", "h3T"):
            srcsb = {"agg1": aggT_sb, "h1T": h1T_sb, "nmall": nmall_sb,
                     "h2T": h2T_sb, "h3T": h3T_sb}[dbg]
            dv = feat.tile([128, NPC], F32, name="dbgv")
            nc.vector.tensor_copy(dv[:], srcsb[:])
            nc.sync.dma_start(dbg_t[:], dv[:])
        elif dbg == "h1full":
            hsb = feat.tile([128, NPC], BF16, name="dbgh")
            nc.sync.dma_start(
                hsb[:], h1_full[:NPC].rearrange("(b p) f -> p b f", p=128))
            dv = feat.tile([128, NPC], F32, name="dbgv")
            nc.vector.tensor_copy(dv[:], hsb[:])
            nc.sync.dma_start(dbg_t[:], dv[:])

        # ---- pooling: pooledT[h, g] = sum_n h3[n, h] * (batch[n] == g) ----
        pool_ps = psP.tile([H, G], F32)
        for b in range(NBLK):
            tp = psT.tile([128, 128], BF16, tag="tp", name=f"tpp_{b}")
            nc.tensor.transpose(tp[:], h3T_sb[:, b * 128:(b + 1) * 128],
                                ident_sb[:])
            nm = nm_pool.tile([128, 128], BF16, tag="nm", name=f"nmp_{b}")
            nc.scalar.copy(nm[:], tp[:])
            nc.tensor.matmul(pool_ps[:], nm[:],
                             pm_sb[:, b * 128:(b + 1) * 128],
                             start=(b == 0), stop=(b == NBLK - 1))
        poolT_sb = sm_pool.tile([H, G], F32)
        nc.vector.tensor_copy(poolT_sb[:], pool_ps[:])
        nc.sync.dma_start(pool_in[:], poolT_sb[:])
        nc.gpsimd.collective_compute(
            "AllReduce", ALU.add, replica_groups=[list(range(NCORES))],
            ins=[pool_in.opt()], outs=[pool_out.opt()])
        poolT_full = sm_pool.tile([H, G], F32)
        nc.sync.dma_start(poolT_full[:], pool_out[:])

        # ---- logits = pooled @ w_out + b_out, then log_softmax ----
        log_ps = psB.tile([H, 512], F32, tag="hp", name="log_ps")
        nc.tensor.matmul(log_ps[:G, :C_OUT], poolT_full[:], wout_sb[:],
                         start=True, stop=False)
        nc.tensor.matmul(log_ps[:G, :C_OUT], ones_sb[:], bout_sb[:],
                         start=False, stop=True)
        logits = sm_pool.tile([G, C_OUT], F32)
        nc.vector.tensor_copy(logits[:], log_ps[:G, :C_OUT])
        mx = sm_pool.tile([G, 1], F32)
        nc.vector.tensor_reduce(mx[:], logits[:], mybir.AxisListType.X,
                                ALU.max)
        negmx = sm_pool.tile([G, 1], F32)
        nc.scalar.mul(negmx[:], mx[:], -1.0)
        expv = sm_pool.tile([G, C_OUT], F32)
        nc.scalar.activation(expv[:], logits[:], AF.Exp, bias=negmx[:])
        sm = sm_pool.tile([G, 1], F32)
        nc.vector.tensor_reduce(sm[:], expv[:], mybir.AxisListType.X, ALU.add)
        lse = sm_pool.tile([G, 1], F32)
        nc.scalar.activation(lse[:], sm[:], AF.Ln)
        mxlse = sm_pool.tile([G, 1], F32)
        nc.vector.tensor_add(mxlse[:], mx[:], lse[:])
        outv = sm_pool.tile([G, C_OUT], F32)
        nc.vector.tensor_scalar(outv[:], logits[:], mxlse[:], None,
                                ALU.subtract)
        nc.sync.dma_start(out_t[:], outv[:])

    nc.compile()
    return nc


_CACHE = {}


def _weight_maps(w1_rel, b1, w1_root, w2_rel, b2, w2_root, w3_rel, b3,
                 w3_root, w_out, b_out):
    def pad1(w):
        wp = np.zeros((128, H), np.float32)
        wp[:F_IN] = np.asarray(w, np.float32)
        return wp.astype(BF)

    return {
        "w1_rel": pad1(w1_rel),
        "w1_root": pad1(w1_root),
        "w2_rel": np.asarray(w2_rel, np.float32).astype(BF),
        "w2_root": np.asarray(w2_root, np.float32).astype(BF),
        "w3_rel": np.asarray(w3_rel, np.float32).astype(BF),
        "w3_root": np.asarray(w3_root, np.float32).astype(BF),
        "b1": np.asarray(b1, np.float32).reshape(H, 1),
        "b2": np.asarray(b2, np.float32).reshape(H, 1),
        "b3": np.asarray(b3, np.float32).reshape(H, 1),
        "w_out": np.asarray(w_out, np.float32),
        "b_out": np.asarray(b_out, np.float32).reshape(1, C_OUT),
    }


def kernel(x, edge_index, batch, w1_rel, b1, w1_root, w2_rel, b2, w2_root,
           w3_rel, b3, w3_root, w_out, b_out):
    in_maps, struct = _prep_inputs(x, edge_index, batch)
    weights = _weight_maps(w1_rel, b1, w1_root, w2_rel, b2, w2_root,
                           w3_rel, b3, w3_root, w_out, b_out)
    for m in in_maps:
        m.update(weights)

    if struct not in _CACHE:
        _CACHE[struct] = _build_program(struct)
    nc = _CACHE[struct]
    res = run_bass_kernel_spmd(nc, in_maps, core_ids=list(range(NCORES)))
    return np.asarray(res.results[0]["out"], np.float32)
